# revision 13
# baseline (speedup 1.0000x reference)
"""DiffAttnV2-like fused kernel for Trainium2 (8 NeuronCores).

Sharding: core = 4*b + g  (b = batch 0..1, g = head-group 0..3, 4 heads each).
Each core computes its 4 output heads' attention and a partial out = y_g @ Wo_g;
host sums the 4 partials per batch.

Per-core dataflow (float32r matmuls - full PE rate, ~1.5e-4 rel rounding):
  4 phases over t-columns (512 each):
    projections into transposed layouts (qT/kT [d,t]; v natural [t,d]; lamT)
    causal attention in sT=[tk,tq] layout; ACT exp evacuates PSUM;
    denominator via ones-column matmul; normalize/combine via K=1 broadcast
    matmuls; partial output projection streamed per 512-col group.
"""
import sys
sys.path.insert(0, "/opt/trn_rl_repo")
from contextlib import ExitStack

import numpy as np

from concourse import bacc, mybir, tile
from concourse.bass_utils import run_bass_kernel_spmd

B, T, D, H = 2, 2048, 2048, 16
HPC = 4               # heads per core
NC = 8                # cores
NDC = D // 128        # 16 contraction chunks
NPH = 4               # t-phases
PT = T // NPH         # 512 t-cols per phase
SCALE = 1.0 / float(np.sqrt(D // H))

f32 = mybir.dt.float32
f32r = mybir.dt.float32r
EXP = mybir.ActivationFunctionType.Exp
SIG = mybir.ActivationFunctionType.Sigmoid

_CACHE = {}


def _build():
    nc = bacc.Bacc("TRN2", target_bir_lowering=False, debug=False)
    xTp = nc.dram_tensor("xTp", [NPH, 128, NDC, PT], f32r, kind="ExternalInput").ap()
    wqp = nc.dram_tensor("wqp", [8, 128, NDC, 128], f32r, kind="ExternalInput").ap()
    wkp = nc.dram_tensor("wkp", [HPC, 128, NDC, 128], f32r, kind="ExternalInput").ap()
    wvp = nc.dram_tensor("wvp", [2, 128, NDC, 256], f32r, kind="ExternalInput").ap()
    wlamp = nc.dram_tensor("wlamp", [128, NDC, HPC], f32r, kind="ExternalInput").ap()
    wop = nc.dram_tensor("wop", [4, 128, HPC, 512], f32r, kind="ExternalInput").ap()
    mstrip = nc.dram_tensor("mstrip", [128, 896], f32r, kind="ExternalInput").ap()
    selin = nc.dram_tensor("selin", [HPC, 512], f32r, kind="ExternalInput").ap()
    idin = nc.dram_tensor("idin", [128, 128], f32r, kind="ExternalInput").ap()
    out = nc.dram_tensor("out", [T, D], f32, kind="ExternalOutput").ap()

    with tile.TileContext(nc) as tc, ExitStack() as ctx:
        ctx.enter_context(nc.allow_low_precision(reason="fp32r matmul pipeline"))
        persist = ctx.enter_context(tc.tile_pool(name="persist", bufs=1))
        xpool = ctx.enter_context(tc.tile_pool(name="xpool", bufs=1))
        qpool = ctx.enter_context(tc.tile_pool(name="qpool", bufs=1))
        wpool = ctx.enter_context(tc.tile_pool(name="wpool", bufs=2))
        epool = ctx.enter_context(tc.tile_pool(name="epool", bufs=4))
        cpool = ctx.enter_context(tc.tile_pool(name="cpool", bufs=1))
        opool = ctx.enter_context(tc.tile_pool(name="opool", bufs=2))
        # PSUM: s4 (4 banks x1) + acc (1x1) + den (1x1) + tr (1x2) = 8 banks
        pps = ctx.enter_context(tc.tile_pool(name="pps", bufs=1, space="PSUM"))
        ppacc = ctx.enter_context(tc.tile_pool(name="ppacc", bufs=1, space="PSUM"))
        ppden = ctx.enter_context(tc.tile_pool(name="ppden", bufs=1, space="PSUM"))
        pptr = ctx.enter_context(tc.tile_pool(name="pptr", bufs=2, space="PSUM"))

        # persistent tensors
        kT = persist.tile([128, HPC, T], f32r)          # 32KB
        vn = persist.tile([128, 2, NDC, 2, 128], f32r)  # 32KB [tk,(pair,tkc,j),d]
        ms = persist.tile([128, 896], f32r)             # 3.5KB
        nc.sync.dma_start(out=ms[:], in_=mstrip[:])
        sel = persist.tile([HPC, HPC, 128], f32r)       # head-row selectors
        nc.sync.dma_start(out=sel.rearrange("p a b -> p (a b)"), in_=selin[:])
        iden = persist.tile([128, 128], f32r)           # identity for mask-add mms
        nc.sync.dma_start(out=iden[:], in_=idin[:])
        ones_col_f = persist.tile([128, 1], f32)
        nc.vector.memset(ones_col_f[:], 1.0)
        ones_col = persist.tile([128, 1], f32r)
        nc.vector.tensor_copy(ones_col[:], ones_col_f[:])
        ones_row_f = persist.tile([1, 128], f32)
        nc.vector.memset(ones_row_f[:], 1.0)
        ones_row = persist.tile([1, 128], f32r)
        nc.vector.tensor_copy(ones_row[:], ones_row_f[:])

        for ph in range(NPH):
            t0 = PT * ph
            # ---- x^T slice for this phase: [128, dc, 512] ----
            xTh = xpool.tile([128, NDC, PT], f32r, name=f"xTh{ph}", tag="xTh")
            nc.sync.dma_start(out=xTh[:], in_=xTp[ph])

            # ---- q projections (8 q-heads: 0..3 from wq1, 4..7 from wq2) ----
            qTh = qpool.tile([128, 8, PT], f32r, name=f"qTh{ph}", tag="qTh")
            for qh in range(8):
                wt = wpool.tile([128, NDC, 128], f32r, name=f"wq{ph}_{qh}", tag="wq")
                nc.sync.dma_start(out=wt[:], in_=wqp[qh])
                ps = pptr.tile([128, PT], f32, name=f"psq{ph}_{qh}", tag="tr")
                for dc in range(NDC):
                    nc.tensor.matmul(ps[:], wt[:, dc], xTh[:, dc],
                                     start=(dc == 0), stop=(dc == NDC - 1))
                nc.vector.tensor_copy(qTh[:, qh], ps[:])

            # ---- k projections (4 k-heads) ----
            for kh in range(HPC):
                wt = wpool.tile([128, NDC, 128], f32r, name=f"wk{ph}_{kh}", tag="wq")
                nc.sync.dma_start(out=wt[:], in_=wkp[kh])
                ps = pptr.tile([128, PT], f32, name=f"psk{ph}_{kh}", tag="tr")
                for dc in range(NDC):
                    nc.tensor.matmul(ps[:], wt[:, dc], xTh[:, dc],
                                     start=(dc == 0), stop=(dc == NDC - 1))
                nc.vector.tensor_copy(kT[:, kh, t0:t0 + PT], ps[:])

            # ---- v projections (2 pairs x 256 cols), natural [tk, d] layout ----
            for pair in range(2):
                wt = wpool.tile([128, NDC, 256], f32r, name=f"wv{ph}_{pair}",
                                tag="wv", bufs=1)
                nc.sync.dma_start(out=wt[:], in_=wvp[pair])
                for tsub in range(4):
                    tkc = 4 * ph + tsub
                    ps = pptr.tile([128, 256], f32, name=f"psv{ph}_{pair}_{tsub}",
                                   tag="tr")
                    for dc in range(NDC):
                        nc.tensor.matmul(
                            ps[:], xTh[:, dc, 128 * tsub:128 * (tsub + 1)],
                            wt[:, dc], start=(dc == 0), stop=(dc == NDC - 1))
                    nc.vector.tensor_copy(
                        vn[:, pair, tkc].rearrange("p a b -> p (a b)"), ps[:])

            # ---- lam projection + sigmoid ----
            wlt = wpool.tile([128, NDC, HPC], f32r, name=f"wl{ph}", tag="wl")
            nc.sync.dma_start(out=wlt[:], in_=wlamp[:])
            psl = pptr.tile([HPC, PT], f32, name=f"psl{ph}", tag="tr")
            for dc in range(NDC):
                nc.tensor.matmul(psl[:], wlt[:, dc], xTh[:, dc],
                                 start=(dc == 0), stop=(dc == NDC - 1))
            lamS = cpool.tile([HPC, PT], f32r, name=f"lam{ph}", tag="lam", bufs=2)
            nc.scalar.activation(lamS[:], psl[:], SIG)

            # ---- attention for tq-group [t0, t0+512), 4 head-pairs ----
            ntk = 4 * (ph + 1)
            nbt = ntk // 4
            yh = qpool.tile([128, HPC, PT], f32r, name=f"yh{ph}", tag="yh")
            for hl in range(HPC):
                t1 = None
                pending = None   # closure: finish j0 combine after j1 starts
                for j, qh in enumerate((hl, 4 + hl)):
                    khl = (hl // 2) if j == 0 else (2 + hl // 2)
                    pair, pj = khl // 2, khl % 2
                    ps_y = ppacc.tile([128, PT], f32, name=f"psy{ph}_{hl}_{j}",
                                      tag="acc")
                    ps_den = ppden.tile([1, PT], f32, name=f"psd{ph}_{hl}_{j}",
                                        tag="den")

                    def consume(bt, exs, ps_y=ps_y, ps_den=ps_den, pair=pair, pj=pj):
                        for c in range(4):
                            tkc = 4 * bt + c
                            ex = exs[c // 2]
                            cc = c % 2
                            nc.tensor.matmul(ps_den[:], ones_col[:], ex[:, cc],
                                             start=(tkc == 0), stop=(tkc == ntk - 1))
                            nc.tensor.matmul(ps_y[:], vn[:, pair, tkc, pj], ex[:, cc],
                                             start=(tkc == 0), stop=(tkc == ntk - 1))

                    prev = None
                    for bt in range(nbt):
                        ps_s = pps.tile([128, 4, PT], f32,
                                        name=f"pss{ph}_{hl}_{j}_{bt}", tag="s4")
                        exs = []
                        for half in range(2):
                            for cc in range(2):
                                c = 2 * half + cc
                                tkc = 4 * bt + c
                                o = 128 * tkc - t0
                                diag = o >= 0
                                nc.tensor.matmul(
                                    ps_s[:, c],
                                    kT[:, khl, 128 * tkc:128 * (tkc + 1)],
                                    qTh[:, qh], start=True, stop=not diag)
                                if diag:   # accumulate -1e30 causal mask strip
                                    nc.tensor.matmul(
                                        ps_s[:, c], iden[:],
                                        ms[:, 384 - o:896 - o],
                                        start=False, stop=True)
                            ex = epool.tile([128, 2, PT], f32r,
                                            name=f"ex{ph}_{hl}_{j}_{bt}_{half}",
                                            tag="ex")
                            nc.scalar.activation(ex[:], ps_s[:, 2 * half:2 * half + 2],
                                                 EXP, scale=SCALE)
                            exs.append(ex)
                        if bt == 0 and pending is not None:
                            pending()   # j0's bcast+normalize overlaps j1 start
                            pending = None
                        if prev is not None:
                            consume(*prev)
                        prev = (bt, exs)
                    consume(*prev)

                    # reciprocal of denominator (frees den bank)
                    rd_f = cpool.tile([1, PT], f32, name=f"rdf{ph}_{hl}_{j}",
                                      tag="rdf")
                    nc.vector.reciprocal_approx_fast(rd_f[:], ps_den[:])
                    rden_j = cpool.tile([1, PT], f32r, name=f"rden{ph}_{hl}_{j}",
                                        tag=f"rden{j}")
                    nc.vector.tensor_copy(rden_j[:], rd_f[:])

                    def combine_j(j=j, ps_y=ps_y, rden_j=rden_j, hl=hl):
                        nonlocal t1
                        ps_b = pptr.tile([128, PT], f32, name=f"psb{ph}_{hl}_{j}",
                                         tag="tr")
                        nc.tensor.matmul(ps_b[:], ones_row[:], rden_j[:],
                                         start=True, stop=True)
                        rB = cpool.tile([128, PT], f32, name=f"rB{ph}_{hl}_{j}",
                                        tag="rB")
                        nc.vector.tensor_copy(rB[:], ps_b[:])
                        if j == 0:
                            t1 = cpool.tile([128, PT], f32, name=f"t1{ph}_{hl}",
                                            tag="t1")
                            nc.vector.tensor_mul(t1[:], ps_y[:], rB[:])
                        else:
                            ps_lam = pptr.tile([128, PT], f32,
                                               name=f"pslam{ph}_{hl}", tag="tr")
                            nc.tensor.matmul(ps_lam[:], sel[:, hl], lamS[:],
                                             start=True, stop=True)
                            t2 = cpool.tile([128, PT], f32, name=f"t2{ph}_{hl}",
                                            tag="t2")
                            nc.vector.tensor_mul(t2[:], ps_y[:], rB[:])
                            nc.vector.tensor_mul(t2[:], t2[:], ps_lam[:])
                            nc.vector.tensor_sub(yh[:, hl], t1[:], t2[:])

                    if j == 0:
                        pending = combine_j
                    else:
                        combine_j()

            # ---- Wo partial: out[t0:t0+512, :] = sum_h yh^T_h @ wo_h ----
            for dout in range(4):
                wo4 = wpool.tile([128, HPC, 512], f32r, name=f"wo{ph}_{dout}",
                                 tag="wo4", bufs=1)
                nc.sync.dma_start(out=wo4[:], in_=wop[dout])
                for tsub in range(4):
                    opool_ps = ppacc if (dout * 4 + tsub) % 2 == 0 else ppden
                    ps_o = opool_ps.tile([128, 512], f32,
                                         name=f"pso{ph}_{dout}_{tsub}",
                                         tag="acc" if (dout * 4 + tsub) % 2 == 0 else "den")
                    for hl in range(HPC):
                        nc.tensor.matmul(
                            ps_o[:], yh[:, hl, 128 * tsub:128 * (tsub + 1)],
                            wo4[:, hl], start=(hl == 0), stop=(hl == HPC - 1))
                    ob = opool.tile([128, 512], f32, name=f"ob{ph}_{dout}_{tsub}",
                                    tag="ob")
                    nc.vector.tensor_copy(ob[:], ps_o[:])
                    nc.sync.dma_start(
                        out=out[t0 + 128 * tsub:t0 + 128 * (tsub + 1),
                                512 * dout:512 * (dout + 1)],
                        in_=ob[:])
    nc.compile()
    return nc


def _get_nc():
    if "nc" not in _CACHE:
        _CACHE["nc"] = _build()
    return _CACHE["nc"]


def kernel(x, Wq1, Wq2, Wk, Wv, Wlam, Wo, **_ignored):
    x = np.ascontiguousarray(np.asarray(x, dtype=np.float32))
    Wq1 = np.asarray(Wq1, dtype=np.float32)
    Wq2 = np.asarray(Wq2, dtype=np.float32)
    Wk = np.asarray(Wk, dtype=np.float32)
    Wv = np.asarray(Wv, dtype=np.float32)
    Wlam = np.asarray(Wlam, dtype=np.float32)
    Wo = np.asarray(Wo, dtype=np.float32)

    cc = np.arange(896)[None, :]
    rr = np.arange(128)[:, None]
    mask = np.where(cc >= rr + 384, 0.0, -1e30).astype(np.float32)
    idv = np.eye(128, dtype=np.float32)
    selv = np.zeros((HPC, HPC, 128), dtype=np.float32)
    for i in range(HPC):
        selv[i, i, :] = 1.0
    selv = selv.reshape(HPC, 512)

    def chunk_cols(w):
        # [D, C] -> [C//128 heads? no: generic [D, C] -> [C/128? ] ] handled per-use
        return w

    xTs = []
    for b in range(B):
        xt = x[b].T                                   # [D, T]
        xTs.append(np.ascontiguousarray(
            xt.reshape(NDC, 128, NPH, PT).transpose(2, 1, 0, 3)))

    in_maps = []
    for core in range(NC):
        b, g = divmod(core, 4)
        kv_cols = np.r_[256 * g:256 * g + 256, 1024 + 256 * g:1024 + 256 * g + 256]
        wq_s = np.concatenate([Wq1[:, 512 * g:512 * (g + 1)],
                               Wq2[:, 512 * g:512 * (g + 1)]], axis=1)  # [D, 1024]
        wqp_v = np.ascontiguousarray(
            wq_s.reshape(NDC, 128, 8, 128).transpose(2, 1, 0, 3))
        wk_s = Wk[:, kv_cols]
        wkp_v = np.ascontiguousarray(
            wk_s.reshape(NDC, 128, HPC, 128).transpose(2, 1, 0, 3))
        wv_s = Wv[:, kv_cols]
        wvp_v = np.ascontiguousarray(
            wv_s.reshape(NDC, 128, 2, 256).transpose(2, 1, 0, 3))
        wlam_s = Wlam[:, 4 * g:4 * (g + 1)]
        wlamp_v = np.ascontiguousarray(
            wlam_s.reshape(NDC, 128, HPC).transpose(1, 0, 2))
        wo_s = Wo[512 * g:512 * (g + 1), :]
        wop_v = np.ascontiguousarray(
            wo_s.reshape(HPC, 128, 4, 512).transpose(2, 1, 0, 3))
        in_maps.append({
            "xTp": xTs[b],
            "wqp": wqp_v,
            "wkp": wkp_v,
            "wvp": wvp_v,
            "wlamp": wlamp_v,
            "wop": wop_v,
            "mstrip": mask,
            "selin": selv,
            "idin": idv,
        })

    res = run_bass_kernel_spmd(_get_nc(), in_maps, list(range(NC)), **_CACHE.get("run_kwargs", {}))
    _CACHE["last_res"] = res
    out = np.zeros((B, T, D), dtype=np.float32)
    for core in range(NC):
        out[core // 4] += res.results[core]["out"]
    return out


# revision 17
# speedup vs baseline: 1.0127x; 1.0127x over previous
"""DiffAttnV2-like fused kernel for Trainium2 (8 NeuronCores).

Sharding: core = 4*b + g  (b = batch 0..1, g = head-group 0..3, 4 heads each).
Each core computes its 4 output heads' attention and a partial out = y_g @ Wo_g;
host sums the 4 partials per batch.

Per-core dataflow (float32r matmuls - full PE rate, ~1.5e-4 rel rounding):
  4 phases over t-columns (512 each):
    projections into transposed layouts (qT/kT [d,t]; v natural [t,d]; lamT)
    causal attention in sT=[tk,tq] layout; ACT exp evacuates PSUM;
    denominator via ones-column matmul; normalize/combine via K=1 broadcast
    matmuls; partial output projection streamed per 512-col group.
"""
import sys
sys.path.insert(0, "/opt/trn_rl_repo")
from contextlib import ExitStack

import numpy as np

from concourse import bacc, mybir, tile
from concourse.bass_utils import run_bass_kernel_spmd

B, T, D, H = 2, 2048, 2048, 16
HPC = 4               # heads per core
NC = 8                # cores
NDC = D // 128        # 16 contraction chunks
NPH = 4               # t-phases
PT = T // NPH         # 512 t-cols per phase
SCALE = 1.0 / float(np.sqrt(D // H))

f32 = mybir.dt.float32
f32r = mybir.dt.float32r
EXP = mybir.ActivationFunctionType.Exp
SIG = mybir.ActivationFunctionType.Sigmoid

_CACHE = {}


def _build():
    nc = bacc.Bacc("TRN2", target_bir_lowering=False, debug=False)
    xTp = nc.dram_tensor("xTp", [NPH, 128, NDC, PT], f32r, kind="ExternalInput").ap()
    wqp = nc.dram_tensor("wqp", [8, 128, NDC, 128], f32r, kind="ExternalInput").ap()
    wkp = nc.dram_tensor("wkp", [HPC, 128, NDC, 128], f32r, kind="ExternalInput").ap()
    wvp = nc.dram_tensor("wvp", [2, 128, NDC, 256], f32r, kind="ExternalInput").ap()
    wlamp = nc.dram_tensor("wlamp", [128, NDC, HPC], f32r, kind="ExternalInput").ap()
    wop = nc.dram_tensor("wop", [4, 128, HPC, 512], f32r, kind="ExternalInput").ap()
    mstrip = nc.dram_tensor("mstrip", [128, 896], f32r, kind="ExternalInput").ap()
    selin = nc.dram_tensor("selin", [HPC, 512], f32r, kind="ExternalInput").ap()
    idin = nc.dram_tensor("idin", [128, 128], f32r, kind="ExternalInput").ap()
    onesin = nc.dram_tensor("onesin", [128, 4], f32r, kind="ExternalInput").ap()
    sel2in = nc.dram_tensor("sel2in", [2, 256], f32r, kind="ExternalInput").ap()
    out = nc.dram_tensor("out", [T, D], f32, kind="ExternalOutput").ap()

    with tile.TileContext(nc) as tc, ExitStack() as ctx:
        ctx.enter_context(nc.allow_low_precision(reason="fp32r matmul pipeline"))
        persist = ctx.enter_context(tc.tile_pool(name="persist", bufs=1))
        xpool = ctx.enter_context(tc.tile_pool(name="xpool", bufs=1))
        qpool = ctx.enter_context(tc.tile_pool(name="qpool", bufs=1))
        wpool = ctx.enter_context(tc.tile_pool(name="wpool", bufs=2))
        epool = ctx.enter_context(tc.tile_pool(name="epool", bufs=2))
        cpool = ctx.enter_context(tc.tile_pool(name="cpool", bufs=1))
        opool = ctx.enter_context(tc.tile_pool(name="opool", bufs=2))
        # PSUM banks: s4 (4x1) + acc (1x2) + den (1x1) + tr (1x1) = 8
        pps = ctx.enter_context(tc.tile_pool(name="pps", bufs=1, space="PSUM"))
        ppacc = ctx.enter_context(tc.tile_pool(name="ppacc", bufs=2, space="PSUM"))
        ppden = ctx.enter_context(tc.tile_pool(name="ppden", bufs=1, space="PSUM"))
        pptr = ctx.enter_context(tc.tile_pool(name="pptr", bufs=1, space="PSUM"))

        kT = persist.tile([128, HPC, T], f32r)          # 32KB
        vn = persist.tile([128, 2, NDC, 2, 128], f32r)  # 32KB [tk,(pair,tkc,j),d]
        ms = persist.tile([128, 896], f32r)             # -1e30/0 causal strip
        nc.sync.dma_start(out=ms[:], in_=mstrip[:])
        sel = persist.tile([HPC, HPC, 128], f32r)       # head-row selectors
        nc.sync.dma_start(out=sel.rearrange("p a b -> p (a b)"), in_=selin[:])
        iden = persist.tile([128, 128], f32r)           # identity for mask-add
        nc.sync.dma_start(out=iden[:], in_=idin[:])
        ones2 = persist.tile([128, 2, 2], f32r)     # den-row selector stationaries
        nc.sync.dma_start(out=ones2.rearrange("p a b -> p (a b)"), in_=onesin[:])
        sel2 = persist.tile([2, 2, 128], f32r)      # den-row broadcast selectors
        nc.sync.dma_start(out=sel2.rearrange("p a b -> p (a b)"), in_=sel2in[:])
        ones_row_f = persist.tile([1, 128], f32)
        nc.vector.memset(ones_row_f[:], 1.0)
        ones_row = persist.tile([1, 128], f32r)
        nc.vector.tensor_copy(ones_row[:], ones_row_f[:])

        for ph in range(NPH):
            t0 = PT * ph
            # ---- x^T slice for this phase ----
            xTh = xpool.tile([128, NDC, PT], f32r, name=f"xTh{ph}", tag="xTh")
            nc.sync.dma_start(out=xTh[:], in_=xTp[ph])

            # ---- q projections ----
            qTh = qpool.tile([128, 8, PT], f32r, name=f"qTh{ph}", tag="qTh")
            for qh in range(8):
                wt = wpool.tile([128, NDC, 128], f32r, name=f"wq{ph}_{qh}", tag="wq")
                nc.sync.dma_start(out=wt[:], in_=wqp[qh])
                ps = pptr.tile([128, PT], f32, name=f"psq{ph}_{qh}", tag="tr")
                for dc in range(NDC):
                    nc.tensor.matmul(ps[:], wt[:, dc], xTh[:, dc],
                                     start=(dc == 0), stop=(dc == NDC - 1))
                nc.vector.tensor_copy(qTh[:, qh], ps[:])

            # ---- k projections ----
            for kh in range(HPC):
                wt = wpool.tile([128, NDC, 128], f32r, name=f"wk{ph}_{kh}", tag="wq")
                nc.sync.dma_start(out=wt[:], in_=wkp[kh])
                ps = pptr.tile([128, PT], f32, name=f"psk{ph}_{kh}", tag="tr")
                for dc in range(NDC):
                    nc.tensor.matmul(ps[:], wt[:, dc], xTh[:, dc],
                                     start=(dc == 0), stop=(dc == NDC - 1))
                nc.vector.tensor_copy(kT[:, kh, t0:t0 + PT], ps[:])

            # ---- v projections (natural [tk, d]) ----
            for pair in range(2):
                wt = wpool.tile([128, NDC, 256], f32r, name=f"wv{ph}_{pair}",
                                tag="wv", bufs=1)
                nc.sync.dma_start(out=wt[:], in_=wvp[pair])
                for tsub in range(4):
                    tkc = 4 * ph + tsub
                    ps = pptr.tile([128, 256], f32, name=f"psv{ph}_{pair}_{tsub}",
                                   tag="tr")
                    for dc in range(NDC):
                        nc.tensor.matmul(
                            ps[:], xTh[:, dc, 128 * tsub:128 * (tsub + 1)],
                            wt[:, dc], start=(dc == 0), stop=(dc == NDC - 1))
                    nc.vector.tensor_copy(
                        vn[:, pair, tkc].rearrange("p a b -> p (a b)"), ps[:])

            # ---- lam projection + sigmoid ----
            wlt = wpool.tile([128, NDC, HPC], f32r, name=f"wl{ph}", tag="wl")
            nc.sync.dma_start(out=wlt[:], in_=wlamp[:])
            psl = pptr.tile([HPC, PT], f32, name=f"psl{ph}", tag="tr")
            for dc in range(NDC):
                nc.tensor.matmul(psl[:], wlt[:, dc], xTh[:, dc],
                                 start=(dc == 0), stop=(dc == NDC - 1))
            lamS = cpool.tile([HPC, PT], f32r, name=f"lam{ph}", tag="lam", bufs=2)
            nc.scalar.activation(lamS[:], psl[:], SIG)

            # ---- attention: 4 head-pairs, j0/j1 interleaved through one s-tile ----
            ntk = 4 * (ph + 1)
            yh = qpool.tile([128, HPC, PT], f32r, name=f"yh{ph}", tag="yh")
            for hl in range(HPC):
                meta = []
                for j, qh in ((0, hl), (1, 4 + hl)):
                    khl = (hl // 2) if j == 0 else (2 + hl // 2)
                    meta.append((qh, khl, khl // 2, khl % 2))
                ps_y = [ppacc.tile([128, PT], f32, name=f"psy{ph}_{hl}_{j}",
                                   tag="acc") for j in range(2)]
                ps_den = ppden.tile([2, PT], f32, name=f"psd{ph}_{hl}", tag="den")

                def consume(bt, ex):
                    for j in range(2):
                        _, _, pair, pj = meta[j]
                        for cc in range(2):
                            tkc = 2 * bt + cc
                            c = 2 * j + cc
                            nc.tensor.matmul(ps_den[0:2, :], ones2[:, j],
                                             ex[:, c],
                                             start=(j == 0 and tkc == 0),
                                             stop=(j == 1 and tkc == ntk - 1))
                            nc.tensor.matmul(ps_y[j][:], vn[:, pair, tkc, pj],
                                             ex[:, c],
                                             start=(tkc == 0), stop=(tkc == ntk - 1))

                prev = None
                for bt in range(ntk // 2):
                    ps_s = pps.tile([128, 4, PT], f32, name=f"pss{ph}_{hl}_{bt}",
                                    tag="s4")
                    for j in range(2):
                        qh, khl = meta[j][0], meta[j][1]
                        for cc in range(2):
                            tkc = 2 * bt + cc
                            c = 2 * j + cc
                            o = 128 * tkc - t0
                            diag = o >= 0
                            nc.tensor.matmul(
                                ps_s[:, c],
                                kT[:, khl, 128 * tkc:128 * (tkc + 1)],
                                qTh[:, qh], start=True, stop=not diag)
                            if diag:
                                nc.tensor.matmul(ps_s[:, c], iden[:],
                                                 ms[:, 384 - o:896 - o],
                                                 start=False, stop=True)
                    ex = epool.tile([128, 4, PT], f32r, name=f"ex{ph}_{hl}_{bt}",
                                    tag="ex")
                    nc.scalar.activation(ex[:], ps_s[:], EXP, scale=SCALE)
                    if prev is not None:
                        consume(*prev)
                    prev = (bt, ex)
                consume(*prev)

                # combine y_h = y0*r0 - lam_h*(r1*y1)
                t12 = []
                rd_f = cpool.tile([2, PT], f32, name=f"rdf{ph}_{hl}", tag="rdf")
                nc.vector.reciprocal_approx_fast(rd_f[:], ps_den[0:2, :])
                rden2 = cpool.tile([2, PT], f32r, name=f"rden{ph}_{hl}", tag="rden")
                nc.vector.tensor_copy(rden2[:], rd_f[:])
                for j in range(2):
                    ps_b = pptr.tile([128, PT], f32, name=f"psb{ph}_{hl}_{j}",
                                     tag="tr")
                    nc.tensor.matmul(ps_b[:], sel2[:, j], rden2[0:2, :],
                                     start=True, stop=True)
                    rB = cpool.tile([128, PT], f32, name=f"rB{ph}_{hl}_{j}",
                                    tag=f"rB{j}")
                    nc.vector.tensor_copy(rB[:], ps_b[:])
                    tj = cpool.tile([128, PT], f32, name=f"t{j}_{ph}_{hl}",
                                    tag=f"t{j}")
                    nc.vector.tensor_mul(tj[:], ps_y[j][:], rB[:])
                    t12.append(tj)
                ps_lam = pptr.tile([128, PT], f32, name=f"pslam{ph}_{hl}", tag="tr")
                nc.tensor.matmul(ps_lam[:], sel[:, hl], lamS[:],
                                 start=True, stop=True)
                nc.vector.tensor_mul(t12[1][:], t12[1][:], ps_lam[:])
                nc.vector.tensor_sub(yh[:, hl], t12[0][:], t12[1][:])

            # ---- Wo partial ----
            for dout in range(4):
                wo4 = wpool.tile([128, HPC, 512], f32r, name=f"wo{ph}_{dout}",
                                 tag="wo4", bufs=1)
                nc.sync.dma_start(out=wo4[:], in_=wop[dout])
                for tsub in range(4):
                    alt = (dout * 4 + tsub) % 2
                    opl = ppacc if alt == 0 else ppden
                    ps_o = opl.tile([128, 512], f32, name=f"pso{ph}_{dout}_{tsub}",
                                    tag="acc" if alt == 0 else "den")
                    for hl in range(HPC):
                        nc.tensor.matmul(
                            ps_o[:], yh[:, hl, 128 * tsub:128 * (tsub + 1)],
                            wo4[:, hl], start=(hl == 0), stop=(hl == HPC - 1))
                    ob = opool.tile([128, 512], f32, name=f"ob{ph}_{dout}_{tsub}",
                                    tag="ob")
                    nc.vector.tensor_copy(ob[:], ps_o[:])
                    nc.sync.dma_start(
                        out=out[t0 + 128 * tsub:t0 + 128 * (tsub + 1),
                                512 * dout:512 * (dout + 1)],
                        in_=ob[:])
    nc.compile()
    return nc


def _get_nc():
    if "nc" not in _CACHE:
        _CACHE["nc"] = _build()
    return _CACHE["nc"]


def kernel(x, Wq1, Wq2, Wk, Wv, Wlam, Wo, **_ignored):
    x = np.ascontiguousarray(np.asarray(x, dtype=np.float32))
    Wq1 = np.asarray(Wq1, dtype=np.float32)
    Wq2 = np.asarray(Wq2, dtype=np.float32)
    Wk = np.asarray(Wk, dtype=np.float32)
    Wv = np.asarray(Wv, dtype=np.float32)
    Wlam = np.asarray(Wlam, dtype=np.float32)
    Wo = np.asarray(Wo, dtype=np.float32)

    cc = np.arange(896)[None, :]
    rr = np.arange(128)[:, None]
    mask = np.where(cc >= rr + 384, 0.0, -1e30).astype(np.float32)
    idv = np.eye(128, dtype=np.float32)
    ones2 = np.zeros((128, 2, 2), dtype=np.float32)
    ones2[:, 0, 0] = 1.0
    ones2[:, 1, 1] = 1.0
    ones2 = ones2.reshape(128, 4)
    sel2 = np.zeros((2, 2, 128), dtype=np.float32)
    sel2[0, 0, :] = 1.0
    sel2[1, 1, :] = 1.0
    sel2 = sel2.reshape(2, 256)
    selv = np.zeros((HPC, HPC, 128), dtype=np.float32)
    for i in range(HPC):
        selv[i, i, :] = 1.0
    selv = selv.reshape(HPC, 512)

    def chunk_cols(w):
        # [D, C] -> [C//128 heads? no: generic [D, C] -> [C/128? ] ] handled per-use
        return w

    xTs = []
    for b in range(B):
        xt = x[b].T                                   # [D, T]
        xTs.append(np.ascontiguousarray(
            xt.reshape(NDC, 128, NPH, PT).transpose(2, 1, 0, 3)))

    in_maps = []
    for core in range(NC):
        b, g = divmod(core, 4)
        kv_cols = np.r_[256 * g:256 * g + 256, 1024 + 256 * g:1024 + 256 * g + 256]
        wq_s = np.concatenate([Wq1[:, 512 * g:512 * (g + 1)],
                               Wq2[:, 512 * g:512 * (g + 1)]], axis=1)  # [D, 1024]
        wqp_v = np.ascontiguousarray(
            wq_s.reshape(NDC, 128, 8, 128).transpose(2, 1, 0, 3))
        wk_s = Wk[:, kv_cols]
        wkp_v = np.ascontiguousarray(
            wk_s.reshape(NDC, 128, HPC, 128).transpose(2, 1, 0, 3))
        wv_s = Wv[:, kv_cols]
        wvp_v = np.ascontiguousarray(
            wv_s.reshape(NDC, 128, 2, 256).transpose(2, 1, 0, 3))
        wlam_s = Wlam[:, 4 * g:4 * (g + 1)]
        wlamp_v = np.ascontiguousarray(
            wlam_s.reshape(NDC, 128, HPC).transpose(1, 0, 2))
        wo_s = Wo[512 * g:512 * (g + 1), :]
        wop_v = np.ascontiguousarray(
            wo_s.reshape(HPC, 128, 4, 512).transpose(2, 1, 0, 3))
        in_maps.append({
            "xTp": xTs[b],
            "wqp": wqp_v,
            "wkp": wkp_v,
            "wvp": wvp_v,
            "wlamp": wlamp_v,
            "wop": wop_v,
            "mstrip": mask,
            "selin": selv,
            "idin": idv,
            "onesin": ones2,
            "sel2in": sel2,
        })

    res = run_bass_kernel_spmd(_get_nc(), in_maps, list(range(NC)), **_CACHE.get("run_kwargs", {}))
    _CACHE["last_res"] = res
    out = np.zeros((B, T, D), dtype=np.float32)
    for core in range(NC):
        out[core // 4] += res.results[core]["out"]
    return out


# revision 19
# speedup vs baseline: 1.0215x; 1.0087x over previous
"""DiffAttnV2-like fused kernel for Trainium2 (8 NeuronCores).

Sharding: core = 4*b + g  (b = batch 0..1, g = head-group 0..3, 4 heads each).
Each core computes its 4 output heads' attention and a partial out = y_g @ Wo_g;
host sums the 4 partials per batch.

Per-core dataflow (float32r matmuls - full PE rate, ~1.5e-4 rel rounding):
  4 phases over t-columns (512 each):
    projections into transposed layouts (qT/kT [d,t]; v natural [t,d]; lamT)
    causal attention in sT=[tk,tq] layout; ACT exp evacuates PSUM;
    denominator via ones-column matmul; normalize/combine via K=1 broadcast
    matmuls; partial output projection streamed per 512-col group.
"""
import sys
sys.path.insert(0, "/opt/trn_rl_repo")
from contextlib import ExitStack

import numpy as np

from concourse import bacc, mybir, tile
from concourse.bass_utils import run_bass_kernel_spmd

B, T, D, H = 2, 2048, 2048, 16
HPC = 4               # heads per core
NC = 8                # cores
NDC = D // 128        # 16 contraction chunks
NPH = 4               # t-phases
PT = T // NPH         # 512 t-cols per phase
SCALE = 1.0 / float(np.sqrt(D // H))

f32 = mybir.dt.float32
f32r = mybir.dt.float32r
EXP = mybir.ActivationFunctionType.Exp
SIG = mybir.ActivationFunctionType.Sigmoid

_CACHE = {}


def _build():
    nc = bacc.Bacc("TRN2", target_bir_lowering=False, debug=False)
    xTp = nc.dram_tensor("xTp", [NPH, 128, NDC, PT], f32r, kind="ExternalInput").ap()
    wqp = nc.dram_tensor("wqp", [8, 128, NDC, 128], f32r, kind="ExternalInput").ap()
    wkp = nc.dram_tensor("wkp", [HPC, 128, NDC, 128], f32r, kind="ExternalInput").ap()
    wvp = nc.dram_tensor("wvp", [2, 128, NDC, 256], f32r, kind="ExternalInput").ap()
    wlamp = nc.dram_tensor("wlamp", [128, NDC, HPC], f32r, kind="ExternalInput").ap()
    wop = nc.dram_tensor("wop", [4, 128, HPC, 512], f32r, kind="ExternalInput").ap()
    mstrip = nc.dram_tensor("mstrip", [128, 896], f32r, kind="ExternalInput").ap()
    selin = nc.dram_tensor("selin", [HPC, 512], f32r, kind="ExternalInput").ap()
    idin = nc.dram_tensor("idin", [128, 128], f32r, kind="ExternalInput").ap()
    onesin = nc.dram_tensor("onesin", [128, 4], f32r, kind="ExternalInput").ap()
    sel2in = nc.dram_tensor("sel2in", [2, 256], f32r, kind="ExternalInput").ap()
    out = nc.dram_tensor("out", [T, D], f32, kind="ExternalOutput").ap()

    with tile.TileContext(nc) as tc, ExitStack() as ctx:
        ctx.enter_context(nc.allow_low_precision(reason="fp32r matmul pipeline"))
        persist = ctx.enter_context(tc.tile_pool(name="persist", bufs=1))
        xpool = ctx.enter_context(tc.tile_pool(name="xpool", bufs=1))
        qpool = ctx.enter_context(tc.tile_pool(name="qpool", bufs=1))
        wpool = ctx.enter_context(tc.tile_pool(name="wpool", bufs=2))
        epool = ctx.enter_context(tc.tile_pool(name="epool", bufs=2))
        cpool = ctx.enter_context(tc.tile_pool(name="cpool", bufs=1))
        opool = ctx.enter_context(tc.tile_pool(name="opool", bufs=2))
        # PSUM banks: s4 (4x1) + acc (1x2) + den (1x1) + tr (1x1) = 8
        pps = ctx.enter_context(tc.tile_pool(name="pps", bufs=1, space="PSUM"))
        ppacc = ctx.enter_context(tc.tile_pool(name="ppacc", bufs=2, space="PSUM"))
        ppden = ctx.enter_context(tc.tile_pool(name="ppden", bufs=1, space="PSUM"))
        pptr = ctx.enter_context(tc.tile_pool(name="pptr", bufs=1, space="PSUM"))

        kT = persist.tile([128, HPC, T], f32r)          # 32KB
        vn = persist.tile([128, 2, NDC, 2, 128], f32r)  # 32KB [tk,(pair,tkc,j),d]
        ms = persist.tile([128, 896], f32r)             # -1e30/0 causal strip
        nc.sync.dma_start(out=ms[:], in_=mstrip[:])
        sel = persist.tile([HPC, HPC, 128], f32r)       # head-row selectors
        nc.sync.dma_start(out=sel.rearrange("p a b -> p (a b)"), in_=selin[:])
        iden = persist.tile([128, 128], f32r)           # identity for mask-add
        nc.sync.dma_start(out=iden[:], in_=idin[:])
        ones2 = persist.tile([128, 2, 2], f32r)     # den-row selector stationaries
        nc.sync.dma_start(out=ones2.rearrange("p a b -> p (a b)"), in_=onesin[:])
        sel2 = persist.tile([2, 2, 128], f32r)      # den-row broadcast selectors
        nc.sync.dma_start(out=sel2.rearrange("p a b -> p (a b)"), in_=sel2in[:])
        ones_row_f = persist.tile([1, 128], f32)
        nc.vector.memset(ones_row_f[:], 1.0)
        ones_row = persist.tile([1, 128], f32r)
        nc.vector.tensor_copy(ones_row[:], ones_row_f[:])

        for ph in range(NPH):
            t0 = PT * ph
            # ---- x^T slice for this phase ----
            xTh = xpool.tile([128, NDC, PT], f32r, name=f"xTh{ph}", tag="xTh")
            nc.sync.dma_start(out=xTh[:], in_=xTp[ph])

            # ---- q projections ----
            qTh = qpool.tile([128, 8, PT], f32r, name=f"qTh{ph}", tag="qTh")
            for qh in range(8):
                wt = wpool.tile([128, NDC, 128], f32r, name=f"wq{ph}_{qh}", tag="wq")
                nc.sync.dma_start(out=wt[:], in_=wqp[qh])
                ps = pptr.tile([128, PT], f32, name=f"psq{ph}_{qh}", tag="tr")
                for dc in range(NDC):
                    nc.tensor.matmul(ps[:], wt[:, dc], xTh[:, dc],
                                     start=(dc == 0), stop=(dc == NDC - 1))
                nc.vector.tensor_copy(qTh[:, qh], ps[:])

            # ---- k projections ----
            for kh in range(HPC):
                wt = wpool.tile([128, NDC, 128], f32r, name=f"wk{ph}_{kh}", tag="wq")
                nc.sync.dma_start(out=wt[:], in_=wkp[kh])
                ps = pptr.tile([128, PT], f32, name=f"psk{ph}_{kh}", tag="tr")
                for dc in range(NDC):
                    nc.tensor.matmul(ps[:], wt[:, dc], xTh[:, dc],
                                     start=(dc == 0), stop=(dc == NDC - 1))
                nc.vector.tensor_copy(kT[:, kh, t0:t0 + PT], ps[:])

            # ---- v projections (natural [tk, d]) ----
            for pair in range(2):
                wt = wpool.tile([128, NDC, 256], f32r, name=f"wv{ph}_{pair}",
                                tag="wv", bufs=1)
                nc.sync.dma_start(out=wt[:], in_=wvp[pair])
                for tsub in range(4):
                    tkc = 4 * ph + tsub
                    ps = pptr.tile([128, 256], f32, name=f"psv{ph}_{pair}_{tsub}",
                                   tag="tr")
                    for dc in range(NDC):
                        nc.tensor.matmul(
                            ps[:], xTh[:, dc, 128 * tsub:128 * (tsub + 1)],
                            wt[:, dc], start=(dc == 0), stop=(dc == NDC - 1))
                    nc.vector.tensor_copy(
                        vn[:, pair, tkc].rearrange("p a b -> p (a b)"), ps[:])

            # ---- lam projection + sigmoid ----
            wlt = wpool.tile([128, NDC, HPC], f32r, name=f"wl{ph}", tag="wl")
            nc.sync.dma_start(out=wlt[:], in_=wlamp[:])
            psl = pptr.tile([HPC, PT], f32, name=f"psl{ph}", tag="tr")
            for dc in range(NDC):
                nc.tensor.matmul(psl[:], wlt[:, dc], xTh[:, dc],
                                 start=(dc == 0), stop=(dc == NDC - 1))
            lamS = cpool.tile([HPC, PT], f32r, name=f"lam{ph}", tag="lam", bufs=2)
            nc.scalar.activation(lamS[:], psl[:], SIG)

            # ---- attention: 4 head-pairs, j0/j1 interleaved through one s-tile ----
            ntk = 4 * (ph + 1)
            yh = qpool.tile([128, HPC, PT], f32r, name=f"yh{ph}", tag="yh")
            pending_combine = None
            for hl in range(HPC):
                meta = []
                for j, qh in ((0, hl), (1, 4 + hl)):
                    khl = (hl // 2) if j == 0 else (2 + hl // 2)
                    meta.append((qh, khl, khl // 2, khl % 2))
                ps_y = [ppacc.tile([128, PT], f32, name=f"psy{ph}_{hl}_{j}",
                                   tag="acc") for j in range(2)]
                ps_den = ppden.tile([2, PT], f32, name=f"psd{ph}_{hl}", tag="den")

                def consume(bt, ex):
                    for j in range(2):
                        _, _, pair, pj = meta[j]
                        for cc in range(2):
                            tkc = 2 * bt + cc
                            c = 2 * j + cc
                            nc.tensor.matmul(ps_den[0:2, :], ones2[:, j],
                                             ex[:, c],
                                             start=(j == 0 and tkc == 0),
                                             stop=(j == 1 and tkc == ntk - 1))
                            nc.tensor.matmul(ps_y[j][:], vn[:, pair, tkc, pj],
                                             ex[:, c],
                                             start=(tkc == 0), stop=(tkc == ntk - 1))

                prev = None
                for bt in range(ntk // 2):
                    if bt == 1 and pending_combine is not None:
                        pending_combine()
                        pending_combine = None
                    ps_s = pps.tile([128, 4, PT], f32, name=f"pss{ph}_{hl}_{bt}",
                                    tag="s4")
                    for j in range(2):
                        qh, khl = meta[j][0], meta[j][1]
                        for cc in range(2):
                            tkc = 2 * bt + cc
                            c = 2 * j + cc
                            o = 128 * tkc - t0
                            diag = o >= 0
                            nc.tensor.matmul(
                                ps_s[:, c],
                                kT[:, khl, 128 * tkc:128 * (tkc + 1)],
                                qTh[:, qh], start=True, stop=not diag)
                            if diag:
                                nc.tensor.matmul(ps_s[:, c], iden[:],
                                                 ms[:, 384 - o:896 - o],
                                                 start=False, stop=True)
                    ex = epool.tile([128, 4, PT], f32r, name=f"ex{ph}_{hl}_{bt}",
                                    tag="ex")
                    nc.scalar.activation(ex[:], ps_s[:], EXP, scale=SCALE)
                    if prev is not None:
                        consume(*prev)
                    prev = (bt, ex)
                consume(*prev)

                # combine y_h = y0*r0 - lam_h*(r1*y1); deferred to overlap
                rd_f = cpool.tile([2, PT], f32, name=f"rdf{ph}_{hl}", tag="rdf")
                nc.vector.reciprocal_approx_fast(rd_f[:], ps_den[0:2, :])
                rden2 = cpool.tile([2, PT], f32r, name=f"rden{ph}_{hl}", tag="rden")
                nc.vector.tensor_copy(rden2[:], rd_f[:])

                def _combine(hl=hl, ps_y=ps_y, rden2=rden2):
                    t12 = []
                    for j in range(2):
                        ps_b = pptr.tile([128, PT], f32, name=f"psb{ph}_{hl}_{j}",
                                         tag="tr")
                        nc.tensor.matmul(ps_b[:], sel2[:, j], rden2[0:2, :],
                                         start=True, stop=True)
                        rB = cpool.tile([128, PT], f32, name=f"rB{ph}_{hl}_{j}",
                                        tag=f"rB{j}")
                        nc.vector.tensor_copy(rB[:], ps_b[:])
                        tj = cpool.tile([128, PT], f32, name=f"t{j}_{ph}_{hl}",
                                        tag=f"t{j}")
                        nc.vector.tensor_mul(tj[:], ps_y[j][:], rB[:])
                        t12.append(tj)
                    ps_lam = pptr.tile([128, PT], f32, name=f"pslam{ph}_{hl}",
                                       tag="tr")
                    nc.tensor.matmul(ps_lam[:], sel[:, hl], lamS[:],
                                     start=True, stop=True)
                    nc.vector.tensor_mul(t12[1][:], t12[1][:], ps_lam[:])
                    nc.vector.tensor_sub(yh[:, hl], t12[0][:], t12[1][:])

                if hl < HPC - 1 and ntk >= 4:
                    pending_combine = _combine
                else:
                    _combine()

            # ---- Wo partial ----
            for dout in range(4):
                wo4 = wpool.tile([128, HPC, 512], f32r, name=f"wo{ph}_{dout}",
                                 tag="wo4", bufs=1)
                nc.sync.dma_start(out=wo4[:], in_=wop[dout])
                for tsub in range(4):
                    alt = (dout * 4 + tsub) % 3
                    opl = ppacc if alt < 2 else ppden
                    ps_o = opl.tile([128, 512], f32, name=f"pso{ph}_{dout}_{tsub}",
                                    tag="acc" if alt < 2 else "den")
                    for hl in range(HPC):
                        nc.tensor.matmul(
                            ps_o[:], yh[:, hl, 128 * tsub:128 * (tsub + 1)],
                            wo4[:, hl], start=(hl == 0), stop=(hl == HPC - 1))
                    ob = opool.tile([128, 512], f32, name=f"ob{ph}_{dout}_{tsub}",
                                    tag="ob")
                    if (dout * 4 + tsub) % 2 == 0:
                        nc.vector.tensor_copy(ob[:], ps_o[:])
                    else:
                        nc.scalar.copy(ob[:], ps_o[:])
                    nc.sync.dma_start(
                        out=out[t0 + 128 * tsub:t0 + 128 * (tsub + 1),
                                512 * dout:512 * (dout + 1)],
                        in_=ob[:])
    nc.compile()
    return nc


def _get_nc():
    if "nc" not in _CACHE:
        _CACHE["nc"] = _build()
    return _CACHE["nc"]


def kernel(x, Wq1, Wq2, Wk, Wv, Wlam, Wo, **_ignored):
    x = np.ascontiguousarray(np.asarray(x, dtype=np.float32))
    Wq1 = np.asarray(Wq1, dtype=np.float32)
    Wq2 = np.asarray(Wq2, dtype=np.float32)
    Wk = np.asarray(Wk, dtype=np.float32)
    Wv = np.asarray(Wv, dtype=np.float32)
    Wlam = np.asarray(Wlam, dtype=np.float32)
    Wo = np.asarray(Wo, dtype=np.float32)

    cc = np.arange(896)[None, :]
    rr = np.arange(128)[:, None]
    mask = np.where(cc >= rr + 384, 0.0, -1e30).astype(np.float32)
    idv = np.eye(128, dtype=np.float32)
    ones2 = np.zeros((128, 2, 2), dtype=np.float32)
    ones2[:, 0, 0] = 1.0
    ones2[:, 1, 1] = 1.0
    ones2 = ones2.reshape(128, 4)
    sel2 = np.zeros((2, 2, 128), dtype=np.float32)
    sel2[0, 0, :] = 1.0
    sel2[1, 1, :] = 1.0
    sel2 = sel2.reshape(2, 256)
    selv = np.zeros((HPC, HPC, 128), dtype=np.float32)
    for i in range(HPC):
        selv[i, i, :] = 1.0
    selv = selv.reshape(HPC, 512)

    def chunk_cols(w):
        # [D, C] -> [C//128 heads? no: generic [D, C] -> [C/128? ] ] handled per-use
        return w

    xTs = []
    for b in range(B):
        xt = x[b].T                                   # [D, T]
        xTs.append(np.ascontiguousarray(
            xt.reshape(NDC, 128, NPH, PT).transpose(2, 1, 0, 3)))

    in_maps = []
    for core in range(NC):
        b, g = divmod(core, 4)
        kv_cols = np.r_[256 * g:256 * g + 256, 1024 + 256 * g:1024 + 256 * g + 256]
        wq_s = np.concatenate([Wq1[:, 512 * g:512 * (g + 1)],
                               Wq2[:, 512 * g:512 * (g + 1)]], axis=1)  # [D, 1024]
        wqp_v = np.ascontiguousarray(
            wq_s.reshape(NDC, 128, 8, 128).transpose(2, 1, 0, 3))
        wk_s = Wk[:, kv_cols]
        wkp_v = np.ascontiguousarray(
            wk_s.reshape(NDC, 128, HPC, 128).transpose(2, 1, 0, 3))
        wv_s = Wv[:, kv_cols]
        wvp_v = np.ascontiguousarray(
            wv_s.reshape(NDC, 128, 2, 256).transpose(2, 1, 0, 3))
        wlam_s = Wlam[:, 4 * g:4 * (g + 1)]
        wlamp_v = np.ascontiguousarray(
            wlam_s.reshape(NDC, 128, HPC).transpose(1, 0, 2))
        wo_s = Wo[512 * g:512 * (g + 1), :]
        wop_v = np.ascontiguousarray(
            wo_s.reshape(HPC, 128, 4, 512).transpose(2, 1, 0, 3))
        in_maps.append({
            "xTp": xTs[b],
            "wqp": wqp_v,
            "wkp": wkp_v,
            "wvp": wvp_v,
            "wlamp": wlamp_v,
            "wop": wop_v,
            "mstrip": mask,
            "selin": selv,
            "idin": idv,
            "onesin": ones2,
            "sel2in": sel2,
        })

    res = run_bass_kernel_spmd(_get_nc(), in_maps, list(range(NC)), **_CACHE.get("run_kwargs", {}))
    _CACHE["last_res"] = res
    out = np.zeros((B, T, D), dtype=np.float32)
    for core in range(NC):
        out[core // 4] += res.results[core]["out"]
    return out


# revision 21
# speedup vs baseline: 1.0219x; 1.0003x over previous
"""DiffAttnV2-like fused kernel for Trainium2 (8 NeuronCores).

Sharding: core = 4*b + g  (b = batch 0..1, g = head-group 0..3, 4 heads each).
Each core computes its 4 output heads' attention and a partial out = y_g @ Wo_g;
host sums the 4 partials per batch.

Per-core dataflow (float32r matmuls - full PE rate, ~1.5e-4 rel rounding):
  4 phases over t-columns (512 each):
    projections into transposed layouts (qT/kT [d,t]; v natural [t,d]; lamT)
    causal attention in sT=[tk,tq] layout; ACT exp evacuates PSUM;
    denominator via ones-column matmul; normalize/combine via K=1 broadcast
    matmuls; partial output projection streamed per 512-col group.
"""
import sys
sys.path.insert(0, "/opt/trn_rl_repo")
from contextlib import ExitStack

import numpy as np

from concourse import bacc, mybir, tile
from concourse.bass_utils import run_bass_kernel_spmd

B, T, D, H = 2, 2048, 2048, 16
HPC = 4               # heads per core
NC = 8                # cores
NDC = D // 128        # 16 contraction chunks
NPH = 4               # t-phases
PT = T // NPH         # 512 t-cols per phase
SCALE = 1.0 / float(np.sqrt(D // H))

f32 = mybir.dt.float32
f32r = mybir.dt.float32r
EXP = mybir.ActivationFunctionType.Exp
SIG = mybir.ActivationFunctionType.Sigmoid

_CACHE = {}


def _build():
    nc = bacc.Bacc("TRN2", target_bir_lowering=False, debug=False)
    xTp = nc.dram_tensor("xTp", [NPH, 128, NDC, PT], f32r, kind="ExternalInput").ap()
    wqp = nc.dram_tensor("wqp", [8, 128, NDC, 128], f32r, kind="ExternalInput").ap()
    wkp = nc.dram_tensor("wkp", [HPC, 128, NDC, 128], f32r, kind="ExternalInput").ap()
    wvp = nc.dram_tensor("wvp", [2, 128, NDC, 256], f32r, kind="ExternalInput").ap()
    wlamp = nc.dram_tensor("wlamp", [128, NDC, HPC], f32r, kind="ExternalInput").ap()
    wop = nc.dram_tensor("wop", [4, 128, HPC, 512], f32r, kind="ExternalInput").ap()
    mstrip = nc.dram_tensor("mstrip", [128, 896], f32r, kind="ExternalInput").ap()
    selin = nc.dram_tensor("selin", [HPC, 512], f32r, kind="ExternalInput").ap()
    idin = nc.dram_tensor("idin", [128, 128], f32r, kind="ExternalInput").ap()
    onesin = nc.dram_tensor("onesin", [128, 4], f32r, kind="ExternalInput").ap()
    sel2in = nc.dram_tensor("sel2in", [2, 256], f32r, kind="ExternalInput").ap()
    out = nc.dram_tensor("out", [T, D], f32, kind="ExternalOutput").ap()

    with tile.TileContext(nc) as tc, ExitStack() as ctx:
        ctx.enter_context(nc.allow_low_precision(reason="fp32r matmul pipeline"))
        persist = ctx.enter_context(tc.tile_pool(name="persist", bufs=1))
        xpool = ctx.enter_context(tc.tile_pool(name="xpool", bufs=1))
        qpool = ctx.enter_context(tc.tile_pool(name="qpool", bufs=1))
        wpool = ctx.enter_context(tc.tile_pool(name="wpool", bufs=2))
        epool = ctx.enter_context(tc.tile_pool(name="epool", bufs=2))
        cpool = ctx.enter_context(tc.tile_pool(name="cpool", bufs=1))
        opool = ctx.enter_context(tc.tile_pool(name="opool", bufs=2))
        # PSUM banks: s4 (4x1) + acc (1x2) + den (1x1) + tr (1x1) = 8
        pps = ctx.enter_context(tc.tile_pool(name="pps", bufs=1, space="PSUM"))
        ppacc = ctx.enter_context(tc.tile_pool(name="ppacc", bufs=2, space="PSUM"))
        ppden = ctx.enter_context(tc.tile_pool(name="ppden", bufs=1, space="PSUM"))
        pptr = ctx.enter_context(tc.tile_pool(name="pptr", bufs=1, space="PSUM"))

        kT = persist.tile([128, HPC, T], f32r)          # 32KB
        vn = persist.tile([128, 2, NDC, 2, 128], f32r)  # 32KB [tk,(pair,tkc,j),d]
        ms = persist.tile([128, 896], f32r)             # -1e30/0 causal strip
        nc.sync.dma_start(out=ms[:], in_=mstrip[:])
        sel = persist.tile([HPC, HPC, 128], f32r)       # head-row selectors
        nc.sync.dma_start(out=sel.rearrange("p a b -> p (a b)"), in_=selin[:])
        iden = persist.tile([128, 128], f32r)           # identity for mask-add
        nc.sync.dma_start(out=iden[:], in_=idin[:])
        ones2 = persist.tile([128, 2, 2], f32r)     # den-row selector stationaries
        nc.sync.dma_start(out=ones2.rearrange("p a b -> p (a b)"), in_=onesin[:])
        sel2 = persist.tile([2, 2, 128], f32r)      # den-row broadcast selectors
        nc.sync.dma_start(out=sel2.rearrange("p a b -> p (a b)"), in_=sel2in[:])
        ones_row_f = persist.tile([1, 128], f32)
        nc.vector.memset(ones_row_f[:], 1.0)
        ones_row = persist.tile([1, 128], f32r)
        nc.vector.tensor_copy(ones_row[:], ones_row_f[:])

        for ph in range(NPH):
            t0 = PT * ph
            # ---- x^T slice for this phase ----
            xTh = xpool.tile([128, NDC, PT], f32r, name=f"xTh{ph}", tag="xTh")
            nc.sync.dma_start(out=xTh[:], in_=xTp[ph])

            # ---- q projections ----
            qTh = qpool.tile([128, 8, PT], f32r, name=f"qTh{ph}", tag="qTh")
            for qh in range(8):
                wt = wpool.tile([128, NDC, 128], f32r, name=f"wq{ph}_{qh}", tag="wq")
                nc.sync.dma_start(out=wt[:], in_=wqp[qh])
                ps = pptr.tile([128, PT], f32, name=f"psq{ph}_{qh}", tag="tr")
                for dc in range(NDC):
                    nc.tensor.matmul(ps[:], wt[:, dc], xTh[:, dc],
                                     start=(dc == 0), stop=(dc == NDC - 1))
                nc.vector.tensor_copy(qTh[:, qh], ps[:])

            # ---- k projections ----
            for kh in range(HPC):
                wt = wpool.tile([128, NDC, 128], f32r, name=f"wk{ph}_{kh}", tag="wq")
                nc.sync.dma_start(out=wt[:], in_=wkp[kh])
                ps = pptr.tile([128, PT], f32, name=f"psk{ph}_{kh}", tag="tr")
                for dc in range(NDC):
                    nc.tensor.matmul(ps[:], wt[:, dc], xTh[:, dc],
                                     start=(dc == 0), stop=(dc == NDC - 1))
                nc.vector.tensor_copy(kT[:, kh, t0:t0 + PT], ps[:])

            # ---- v projections (natural [tk, d]) ----
            for pair in range(2):
                wt = wpool.tile([128, NDC, 256], f32r, name=f"wv{ph}_{pair}",
                                tag="wv", bufs=1)
                nc.sync.dma_start(out=wt[:], in_=wvp[pair])
                for tsub in range(4):
                    tkc = 4 * ph + tsub
                    ps = pptr.tile([128, 256], f32, name=f"psv{ph}_{pair}_{tsub}",
                                   tag="tr")
                    for dc in range(NDC):
                        nc.tensor.matmul(
                            ps[:], xTh[:, dc, 128 * tsub:128 * (tsub + 1)],
                            wt[:, dc], start=(dc == 0), stop=(dc == NDC - 1))
                    nc.vector.tensor_copy(
                        vn[:, pair, tkc].rearrange("p a b -> p (a b)"), ps[:])

            # ---- lam projection + sigmoid ----
            wlt = wpool.tile([128, NDC, HPC], f32r, name=f"wl{ph}", tag="wl")
            nc.sync.dma_start(out=wlt[:], in_=wlamp[:])
            psl = pptr.tile([HPC, PT], f32, name=f"psl{ph}", tag="tr")
            for dc in range(NDC):
                nc.tensor.matmul(psl[:], wlt[:, dc], xTh[:, dc],
                                 start=(dc == 0), stop=(dc == NDC - 1))
            lamS = cpool.tile([HPC, PT], f32r, name=f"lam{ph}", tag="lam", bufs=1)
            nc.scalar.activation(lamS[:], psl[:], SIG)

            # ---- attention: 4 head-pairs, j0/j1 interleaved through one s-tile ----
            ntk = 4 * (ph + 1)
            yh = qpool.tile([128, HPC, PT], f32r, name=f"yh{ph}", tag="yh")
            pending_combine = None
            for hl in range(HPC):
                meta = []
                for j, qh in ((0, hl), (1, 4 + hl)):
                    khl = (hl // 2) if j == 0 else (2 + hl // 2)
                    meta.append((qh, khl, khl // 2, khl % 2))
                ps_y = [ppacc.tile([128, PT], f32, name=f"psy{ph}_{hl}_{j}",
                                   tag="acc") for j in range(2)]
                ps_den = ppden.tile([2, PT], f32, name=f"psd{ph}_{hl}", tag="den")

                def consume(bt, exs):
                    for j in range(2):
                        _, _, pair, pj = meta[j]
                        for cc in range(2):
                            tkc = 2 * bt + cc
                            exc = exs[j][:, cc]
                            nc.tensor.matmul(ps_den[0:2, :], ones2[:, j], exc,
                                             start=(j == 0 and tkc == 0),
                                             stop=(j == 1 and tkc == ntk - 1))
                            nc.tensor.matmul(ps_y[j][:], vn[:, pair, tkc, pj], exc,
                                             start=(tkc == 0), stop=(tkc == ntk - 1))

                prev = None
                for bt in range(ntk // 2):
                    if bt == 1 and pending_combine is not None:
                        pending_combine()
                        pending_combine = None
                    ps_s = pps.tile([128, 4, PT], f32, name=f"pss{ph}_{hl}_{bt}",
                                    tag="s4")
                    for j in range(2):
                        qh, khl = meta[j][0], meta[j][1]
                        for cc in range(2):
                            tkc = 2 * bt + cc
                            c = 2 * j + cc
                            o = 128 * tkc - t0
                            diag = o >= 0
                            nc.tensor.matmul(
                                ps_s[:, c],
                                kT[:, khl, 128 * tkc:128 * (tkc + 1)],
                                qTh[:, qh], start=True, stop=not diag)
                            if diag:
                                nc.tensor.matmul(ps_s[:, c], iden[:],
                                                 ms[:, 384 - o:896 - o],
                                                 start=False, stop=True)
                    exA = epool.tile([128, 2, PT], f32r,
                                     name=f"exA{ph}_{hl}_{bt}", tag="ex", bufs=3)
                    nc.scalar.activation(exA[:], ps_s[:, 0:2], EXP, scale=SCALE)
                    exB = epool.tile([128, 2, PT], f32r,
                                     name=f"exB{ph}_{hl}_{bt}", tag="ex", bufs=3)
                    nc.scalar.activation(exB[:], ps_s[:, 2:4], EXP, scale=SCALE)
                    if prev is not None:
                        consume(*prev)
                    prev = (bt, (exA, exB))
                consume(*prev)

                # combine y_h = y0*r0 - lam_h*(r1*y1); deferred to overlap
                rd_f = cpool.tile([2, PT], f32, name=f"rdf{ph}_{hl}", tag="rdf")
                nc.vector.reciprocal_approx_fast(rd_f[:], ps_den[0:2, :])
                rden2 = cpool.tile([2, PT], f32r, name=f"rden{ph}_{hl}", tag="rden")
                nc.scalar.copy(rden2[:], rd_f[:])

                def _combine(hl=hl, ps_y=ps_y, rden2=rden2):
                    t12 = []
                    for j in range(2):
                        ps_b = pptr.tile([128, PT], f32, name=f"psb{ph}_{hl}_{j}",
                                         tag="tr")
                        nc.tensor.matmul(ps_b[:], sel2[:, j], rden2[0:2, :],
                                         start=True, stop=True)
                        rB = cpool.tile([128, PT], f32, name=f"rB{ph}_{hl}_{j}",
                                        tag=f"rB{j}")
                        nc.vector.tensor_copy(rB[:], ps_b[:])
                        tj = cpool.tile([128, PT], f32, name=f"t{j}_{ph}_{hl}",
                                        tag=f"t{j}")
                        nc.vector.tensor_mul(tj[:], ps_y[j][:], rB[:])
                        t12.append(tj)
                    ps_lam = pptr.tile([128, PT], f32, name=f"pslam{ph}_{hl}",
                                       tag="tr")
                    nc.tensor.matmul(ps_lam[:], sel[:, hl], lamS[:],
                                     start=True, stop=True)
                    nc.vector.tensor_mul(t12[1][:], t12[1][:], ps_lam[:])
                    nc.vector.tensor_sub(yh[:, hl], t12[0][:], t12[1][:])

                if hl < HPC - 1 and ntk >= 4:
                    pending_combine = _combine
                else:
                    _combine()

            # ---- Wo partial ----
            for dout in range(4):
                wo4 = wpool.tile([128, HPC, 512], f32r, name=f"wo{ph}_{dout}",
                                 tag="wo4", bufs=2)
                nc.sync.dma_start(out=wo4[:], in_=wop[dout])
                for tsub in range(4):
                    alt = (dout * 4 + tsub) % 3
                    opl = ppacc if alt < 2 else ppden
                    ps_o = opl.tile([128, 512], f32, name=f"pso{ph}_{dout}_{tsub}",
                                    tag="acc" if alt < 2 else "den")
                    for hl in range(HPC):
                        nc.tensor.matmul(
                            ps_o[:], yh[:, hl, 128 * tsub:128 * (tsub + 1)],
                            wo4[:, hl], start=(hl == 0), stop=(hl == HPC - 1))
                    ob = opool.tile([128, 512], f32, name=f"ob{ph}_{dout}_{tsub}",
                                    tag="ob")
                    if (dout * 4 + tsub) % 2 == 0:
                        nc.vector.tensor_copy(ob[:], ps_o[:])
                    else:
                        nc.scalar.copy(ob[:], ps_o[:])
                    nc.sync.dma_start(
                        out=out[t0 + 128 * tsub:t0 + 128 * (tsub + 1),
                                512 * dout:512 * (dout + 1)],
                        in_=ob[:])
    nc.compile()
    return nc


def _get_nc():
    if "nc" not in _CACHE:
        _CACHE["nc"] = _build()
    return _CACHE["nc"]


def kernel(x, Wq1, Wq2, Wk, Wv, Wlam, Wo, **_ignored):
    x = np.ascontiguousarray(np.asarray(x, dtype=np.float32))
    Wq1 = np.asarray(Wq1, dtype=np.float32)
    Wq2 = np.asarray(Wq2, dtype=np.float32)
    Wk = np.asarray(Wk, dtype=np.float32)
    Wv = np.asarray(Wv, dtype=np.float32)
    Wlam = np.asarray(Wlam, dtype=np.float32)
    Wo = np.asarray(Wo, dtype=np.float32)

    cc = np.arange(896)[None, :]
    rr = np.arange(128)[:, None]
    mask = np.where(cc >= rr + 384, 0.0, -1e30).astype(np.float32)
    idv = np.eye(128, dtype=np.float32)
    ones2 = np.zeros((128, 2, 2), dtype=np.float32)
    ones2[:, 0, 0] = 1.0
    ones2[:, 1, 1] = 1.0
    ones2 = ones2.reshape(128, 4)
    sel2 = np.zeros((2, 2, 128), dtype=np.float32)
    sel2[0, 0, :] = 1.0
    sel2[1, 1, :] = 1.0
    sel2 = sel2.reshape(2, 256)
    selv = np.zeros((HPC, HPC, 128), dtype=np.float32)
    for i in range(HPC):
        selv[i, i, :] = 1.0
    selv = selv.reshape(HPC, 512)

    def chunk_cols(w):
        # [D, C] -> [C//128 heads? no: generic [D, C] -> [C/128? ] ] handled per-use
        return w

    xTs = []
    for b in range(B):
        xt = x[b].T                                   # [D, T]
        xTs.append(np.ascontiguousarray(
            xt.reshape(NDC, 128, NPH, PT).transpose(2, 1, 0, 3)))

    in_maps = []
    for core in range(NC):
        b, g = divmod(core, 4)
        kv_cols = np.r_[256 * g:256 * g + 256, 1024 + 256 * g:1024 + 256 * g + 256]
        wq_s = np.concatenate([Wq1[:, 512 * g:512 * (g + 1)],
                               Wq2[:, 512 * g:512 * (g + 1)]], axis=1)  # [D, 1024]
        wqp_v = np.ascontiguousarray(
            wq_s.reshape(NDC, 128, 8, 128).transpose(2, 1, 0, 3))
        wk_s = Wk[:, kv_cols]
        wkp_v = np.ascontiguousarray(
            wk_s.reshape(NDC, 128, HPC, 128).transpose(2, 1, 0, 3))
        wv_s = Wv[:, kv_cols]
        wvp_v = np.ascontiguousarray(
            wv_s.reshape(NDC, 128, 2, 256).transpose(2, 1, 0, 3))
        wlam_s = Wlam[:, 4 * g:4 * (g + 1)]
        wlamp_v = np.ascontiguousarray(
            wlam_s.reshape(NDC, 128, HPC).transpose(1, 0, 2))
        wo_s = Wo[512 * g:512 * (g + 1), :]
        wop_v = np.ascontiguousarray(
            wo_s.reshape(HPC, 128, 4, 512).transpose(2, 1, 0, 3))
        in_maps.append({
            "xTp": xTs[b],
            "wqp": wqp_v,
            "wkp": wkp_v,
            "wvp": wvp_v,
            "wlamp": wlamp_v,
            "wop": wop_v,
            "mstrip": mask,
            "selin": selv,
            "idin": idv,
            "onesin": ones2,
            "sel2in": sel2,
        })

    res = run_bass_kernel_spmd(_get_nc(), in_maps, list(range(NC)), **_CACHE.get("run_kwargs", {}))
    _CACHE["last_res"] = res
    out = np.zeros((B, T, D), dtype=np.float32)
    for core in range(NC):
        out[core // 4] += res.results[core]["out"]
    return out
